# revision 1
# baseline (speedup 1.0000x reference)
"""Trainium2 Bass kernel: MeshGraphNet-style GNN message passing.

Strategy (8 NeuronCores, SPMD):
  - Sort edges by dst. Partition nodes into 128-node blocks; assign a
    contiguous range of blocks to each core. Each core owns all edges whose
    dst falls in its blocks (contiguous in the sorted order).
  - Edge MLPs run feature-major on the PE array ([128 feat x edges]).
  - h[src] is fetched with indirect (gather) DMA from a replicated full
    node-state table, then PE-transposed to feature-major.
  - h[dst] needs no gather: edges are dst-sorted, so h[dst] for a strip is
    h_block^T @ O_T with a one-hot O_T generated on-chip.
  - scatter-mean: edges of one node block accumulate into a PSUM tile via
    one-hot matmuls (O[e, n] = (dst_local[e] == n)). The mean is a
    per-partition scale; the last-layer edge bias is a masked broadcast add.
  - Node-update MLP is per block. Updated node states are AllGathered
    between steps.

Register budgeting: every distinct dynamic-offset DMA expression permanently
consumes address registers on its issuing engine (only SP/Act/Pool can issue
DMAs, ~12 expressions max each). Per-block data is packed so each step loop
needs only 4 dynamic DMA expressions; per-block node states are fetched via
indirect DMA (register-free) using node ids stored in the metadata blob.
"""

import os
import numpy as np

P = 128
USE_F32R = True  # fast fp32 matmul mode for free-dim >= 256 matmuls

LAST = {}


def _ceil_div(a, b):
    return -(-a // b)


def _strips(T):
    out = []
    t0 = 0
    while t0 < T:
        k = min(4, T - t0)
        out.append((t0, k))
        t0 += k
    return out


def prep_host(inputs, n_cores=8):
    """Sort/pad/pack everything on the host."""
    x = np.asarray(inputs["x"], np.float32)
    ea = np.asarray(inputs["edge_attr"], np.float32)
    ei = np.asarray(inputs["edge_index"], np.int32)
    N, NI = x.shape
    E, EI = ea.shape
    L = np.asarray(inputs["ne_W1"]).shape[1]
    OD = np.asarray(inputs["de_W3"]).shape[1]
    S = np.asarray(inputs["pe_W1"]).shape[0]

    NB = _ceil_div(N, P)
    NB = _ceil_div(NB, n_cores) * n_cores
    BPC = NB // n_cores
    N_pad = NB * P

    src = ei[0].astype(np.int64)
    dst = ei[1].astype(np.int64)
    perm = np.argsort(dst, kind="stable")
    src_s = src[perm].astype(np.int32)
    dst_s = dst[perm].astype(np.int32)
    ea_s = ea[perm]

    deg = np.bincount(dst, minlength=N_pad).astype(np.float32)
    inv_deg = (1.0 / np.maximum(deg, 1.0)).astype(np.float32)
    mask = (deg > 0).astype(np.float32)

    block_start = np.searchsorted(dst_s, np.arange(0, N_pad + 1, P))
    cnt = np.diff(block_start)
    T = max(4, int(_ceil_div(cnt.max(), P)))
    E_blk = T * P

    # blob: per-node-block metadata, one row per (block, partition).
    # cols: 0=inv_deg, 1=mask, [2,2+T)=dloc by tile, 2+T=own node id
    # (i32 bits), [3+T,3+2T)=src ids (i32 bits). ownid adjoins src so one
    # indirect DMA gathers the own-block rows and all src rows together.
    C = 3 + 2 * T
    blob = np.zeros((NB, P, C), np.float32)
    blob[:, :, 0] = inv_deg.reshape(NB, P)
    blob[:, :, 1] = mask.reshape(NB, P)
    blob[:, :, 2 + T] = np.arange(N_pad, dtype=np.int32).reshape(
        NB, P).view(np.float32)
    blob[:, :, 2:2 + T] = -1.0
    dlocr = np.full((NB, E_blk), -1.0, np.float32)
    ea_pack = np.zeros((NB, E_blk, EI), np.float32)
    for b in range(NB):
        s0, s1 = int(block_start[b]), int(block_start[b + 1])
        n = s1 - s0
        if n == 0:
            continue
        sl = np.zeros(E_blk, np.int32)
        sl[:n] = src_s[s0:s1]
        blob[b, :, 3 + T:3 + 2 * T] = sl.reshape(T, P).T.view(np.float32)
        sf = np.full(E_blk, -1.0, np.float32)
        sf[:n] = (dst_s[s0:s1] - b * P).astype(np.float32)
        blob[b, :, 2:2 + T] = sf.reshape(T, P).T
        dlocr[b] = sf
        ea_pack[b, :n] = ea_s[s0:s1]

    x_fm = np.zeros((NI, N_pad), np.float32)
    x_fm[:, :N] = x.T

    params = dict(N=N, NI=NI, E=E, EI=EI, L=L, OD=OD, S=S,
                  NB=NB, BPC=BPC, N_pad=N_pad, T=T, E_blk=E_blk, C=C,
                  n_cores=n_cores)

    def wf(name):
        return np.ascontiguousarray(np.asarray(inputs[name], np.float32))

    weights = {
        "ne_W1": wf("ne_W1"), "ne_W2": wf("ne_W2"), "ne_W3": wf("ne_W3"),
        "ee_W1": wf("ee_W1"), "ee_W2": wf("ee_W2"), "ee_W3": wf("ee_W3"),
        "de_W1": wf("de_W1"), "de_W2": wf("de_W2"), "de_W3": wf("de_W3"),
        "pe_W1": wf("pe_W1").reshape(S * 3 * L, L),
        "pe_W2": wf("pe_W2").reshape(S * L, L),
        "pe_W3": wf("pe_W3").reshape(S * L, L),
        "pn_W1": wf("pn_W1").reshape(S * 2 * L, L),
        "pn_W2": wf("pn_W2").reshape(S * L, L),
        "pn_W3": wf("pn_W3").reshape(S * L, L),
        "ne_b1": wf("ne_b1").reshape(L, 1), "ne_b2": wf("ne_b2").reshape(L, 1),
        "ee_b1": wf("ee_b1").reshape(L, 1), "ee_b2": wf("ee_b2").reshape(L, 1),
        "ee_b3": wf("ee_b3").reshape(L, 1),
        "de_b1": wf("de_b1").reshape(L, 1), "de_b2": wf("de_b2").reshape(L, 1),
        "pe_b1": wf("pe_b1").reshape(S * L, 1),
        "pe_b2": wf("pe_b2").reshape(S * L, 1),
        "pn_b1": wf("pn_b1").reshape(S * L, 1),
        "pn_b2": wf("pn_b2").reshape(S * L, 1),
        "ne_b3": wf("ne_b3").reshape(1, L),
        "de_b3": wf("de_b3").reshape(1, OD),
        "pe_b3": wf("pe_b3").reshape(S, L),
        "pn_b3": wf("pn_b3").reshape(S, L),
    }

    in_maps = []
    for c in range(n_cores):
        b0, b1 = c * BPC, (c + 1) * BPC
        m = dict(weights)
        m["xfm"] = x_fm
        m["blob"] = np.ascontiguousarray(blob[b0:b1].reshape(BPC * P, C))
        m["dlocr"] = np.ascontiguousarray(dlocr[b0:b1])
        m["eafm"] = np.ascontiguousarray(
            ea_pack[b0:b1].reshape(BPC * E_blk, EI).T)
        in_maps.append(m)
    return params, in_maps


def build_program(params, debug=False):
    import concourse.bass as bass
    import concourse.bacc as bacc
    import concourse.mybir as mybir
    import concourse.tile as tile
    from concourse.bass import ds, ts
    from concourse.masks import make_identity
    from contextlib import ExitStack

    f32 = mybir.dt.float32
    f32r = mybir.dt.float32r
    i32 = mybir.dt.int32
    Relu = mybir.ActivationFunctionType.Relu
    AO = mybir.AluOpType

    NI, EI, L, OD, S = (params[k] for k in ("NI", "EI", "L", "OD", "S"))
    BPC, N_pad, T, E_blk, C = (params[k] for k in
                               ("BPC", "N_pad", "T", "E_blk", "C"))
    n_cores = params["n_cores"]
    E_cap = BPC * E_blk
    strips = _strips(T)

    fr = f32r if USE_F32R else f32

    def r(ap):  # kept for APs that are already rounded (no-op when disabled)
        return ap

    nc = bacc.Bacc(None, target_bir_lowering=False, debug=debug)

    def par(name, shape, dtype=f32, out=False):
        return nc.declare_dram_parameter(name, list(shape), dtype, isOutput=out)

    xfm_d = par("xfm", [NI, N_pad], fr)
    blob_d = par("blob", [BPC * P, C])
    dlocr_d = par("dlocr", [BPC, E_blk], fr)
    eafm_d = par("eafm", [EI, E_cap], fr)

    w_d = {}
    for nm, shp in [
        ("ne_W1", [NI, L]), ("ne_W2", [L, L]), ("ne_W3", [L, L]),
        ("ee_W1", [EI, L]), ("ee_W2", [L, L]), ("ee_W3", [L, L]),
        ("de_W1", [L, L]), ("de_W2", [L, L]), ("de_W3", [L, OD]),
        ("pe_W1", [S * 3 * L, L]), ("pe_W2", [S * L, L]), ("pe_W3", [S * L, L]),
        ("pn_W1", [S * 2 * L, L]), ("pn_W2", [S * L, L]), ("pn_W3", [S * L, L]),
        ("ne_b1", [L, 1]), ("ne_b2", [L, 1]),
        ("ee_b1", [L, 1]), ("ee_b2", [L, 1]), ("ee_b3", [L, 1]),
        ("de_b1", [L, 1]), ("de_b2", [L, 1]),
        ("pe_b1", [S * L, 1]), ("pe_b2", [S * L, 1]),
        ("pn_b1", [S * L, 1]), ("pn_b2", [S * L, 1]),
        ("ne_b3", [1, L]), ("de_b3", [1, OD]),
        ("pe_b3", [S, L]), ("pn_b3", [S, L]),
    ]:
        w_d[nm] = par(nm, shp)

    out_d = par("out", [BPC * P, OD], out=True)

    h_A = nc.dram_tensor("h_A", [N_pad, L], fr)
    h_B = nc.dram_tensor("h_B", [N_pad, L], fr, addr_space="Shared")
    h_C = nc.dram_tensor("h_C", [N_pad, L], fr, addr_space="Shared")
    h_own = nc.dram_tensor("h_own", [BPC * P, L], fr)
    eblk = nc.dram_tensor("eblk", [BPC * P, E_blk], fr)

    read_buf = [h_A, h_B, h_C, h_B, h_C]
    write_buf = [h_B, h_C, h_B, h_C, None]

    with tile.TileContext(nc) as tc, ExitStack() as ctx:
        wp = ctx.enter_context(tc.tile_pool(name="wp", bufs=1))
        sb2 = ctx.enter_context(tc.tile_pool(name="sb2", bufs=2))
        sb = ctx.enter_context(tc.tile_pool(name="sb", bufs=3))
        sbe = ctx.enter_context(tc.tile_pool(name="sbe", bufs=3))
        sbg = ctx.enter_context(tc.tile_pool(name="sbg", bufs=6))
        sbga = ctx.enter_context(tc.tile_pool(name="sbga", bufs=8))
        sbm = ctx.enter_context(tc.tile_pool(name="sbm", bufs=2))
        ps_s = ctx.enter_context(tc.tile_pool(name="ps_s", bufs=4, space="PSUM"))
        ps_b = ctx.enter_context(tc.tile_pool(name="ps_b", bufs=3, space="PSUM"))
        ps_a = ctx.enter_context(tc.tile_pool(name="ps_a", bufs=1, space="PSUM"))

        identity = wp.tile([P, P], f32, tag="identity")
        make_identity(nc, identity[:])
        iota_i = wp.tile([P, P], i32, tag="iota_i")
        nc.gpsimd.iota(iota_i[:], pattern=[[1, P]], base=0, channel_multiplier=0)
        iota_f = wp.tile([P, P], f32, tag="iota_f")
        nc.vector.tensor_copy(iota_f[:], iota_i[:])
        iotac_i = wp.tile([P, 1], i32, tag="iotac_i")
        nc.gpsimd.iota(iotac_i[:], pattern=[[1, 1]], base=0,
                       channel_multiplier=1)
        iotac_f = wp.tile([P, 1], f32, tag="iotac_f")
        nc.vector.tensor_copy(iotac_f[:], iotac_i[:])
        iotac_b = wp.tile([P, 512], f32, tag="iotac_b")
        nc.vector.tensor_copy(iotac_b[:],
                              iotac_f[:, :1].to_broadcast([P, 512])[:])
        ones_row = wp.tile([1, P], f32, tag="ones_row")
        nc.vector.memset(ones_row[:], 1.0)
        identity_r = wp.tile([P, P], fr, tag="identity_r")
        nc.vector.tensor_copy(identity_r[:], identity[:])
        ones_row_r = wp.tile([1, P], fr, tag="ones_row_r")
        nc.vector.tensor_copy(ones_row_r[:], ones_row[:])

        W = {}

        def load(nm, dram_ap, shape, tag, dt=f32):
            t = wp.tile(list(shape), dt, tag=tag)
            if dt is not f32:
                dram_ap = dram_ap.bitcast(dt)
            nc.sync.dma_start(out=t[:], in_=dram_ap)
            W[nm] = t
            return t

        load("ne_W1", w_d["ne_W1"][:, :], [NI, L], "ne_W1", fr)
        load("ne_W2", w_d["ne_W2"][:, :], [L, L], "ne_W2", fr)
        load("ne_W3", w_d["ne_W3"][:, :], [L, L], "ne_W3", fr)
        load("ee_W1", w_d["ee_W1"][:, :], [EI, L], "ee_W1", fr)
        load("ee_W2", w_d["ee_W2"][:, :], [L, L], "ee_W2", fr)
        load("ee_W3", w_d["ee_W3"][:, :], [L, L], "ee_W3", fr)
        load("de_W1", w_d["de_W1"][:, :], [L, L], "de_W1")
        load("de_W2", w_d["de_W2"][:, :], [L, L], "de_W2")
        load("de_W3", w_d["de_W3"][:, :], [L, OD], "de_W3")
        for nm in ("ne_b1", "ne_b2", "ee_b1", "ee_b2", "ee_b3", "de_b1",
                   "de_b2"):
            load(nm, w_d[nm][:, :], [L, 1], nm)
        load("ne_b3", w_d["ne_b3"][:, :], [1, L], "ne_b3")
        load("de_b3", w_d["de_b3"][:, :], [1, OD], "de_b3")
        for s in range(S):
            for k in range(3):
                load(f"pe_W1_{s}_{k}",
                     w_d["pe_W1"][s * 3 * L + k * L:s * 3 * L + (k + 1) * L, :],
                     [L, L], f"pe_W1_{s}_{k}", fr)
            for k in range(2):
                load(f"pn_W1_{s}_{k}",
                     w_d["pn_W1"][s * 2 * L + k * L:s * 2 * L + (k + 1) * L, :],
                     [L, L], f"pn_W1_{s}_{k}")
            for nm in ("pe_W2", "pe_W3"):
                load(f"{nm}_{s}", w_d[nm][s * L:(s + 1) * L, :], [L, L],
                     f"{nm}_{s}", fr)
            for nm in ("pn_W2", "pn_W3"):
                load(f"{nm}_{s}", w_d[nm][s * L:(s + 1) * L, :], [L, L],
                     f"{nm}_{s}")
            for nm in ("pe_b1", "pe_b2", "pn_b1", "pn_b2"):
                load(f"{nm}_{s}", w_d[nm][s * L:(s + 1) * L, :], [L, 1],
                     f"{nm}_{s}")
            for nm in ("pe_b3", "pn_b3"):
                load(f"{nm}_{s}", w_d[nm][s:s + 1, :], [1, L], f"{nm}_{s}")

        mm = nc.tensor.matmul

        # ---- node encoder: h_A for all nodes (identical on every core) ----
        NCHUNK = N_pad // 512
        with tc.For_i(0, NCHUNK, 1) as c:
            x_t = sb2.tile([NI, 512], fr, tag="x_t")
            nc.gpsimd.dma_start(out=x_t[:], in_=xfm_d[:, ts(c, 512)])
            p1 = ps_b.tile([P, 512], f32, tag="mm_big")
            mm(out=p1[:], lhsT=r(W["ne_W1"][:]), rhs=r(x_t[:]),
               start=True, stop=True)
            a1 = sb2.tile([P, 512], fr, tag="enc_a1")
            nc.scalar.activation(out=a1[:], in_=p1[:], func=Relu,
                                 bias=W["ne_b1"][:, :1])
            p2 = ps_b.tile([P, 512], f32, tag="mm_big")
            mm(out=p2[:], lhsT=r(W["ne_W2"][:]), rhs=r(a1[:]),
               start=True, stop=True)
            a2 = sb2.tile([P, 512], fr, tag="enc_a2")
            nc.scalar.activation(out=a2[:], in_=p2[:], func=Relu,
                                 bias=W["ne_b2"][:, :1])
            for j in range(4):
                p3 = ps_s.tile([P, L], f32, tag="mm_small")
                mm(out=p3[:], lhsT=a2[:, j * P:(j + 1) * P], rhs=W["ne_W3"][:],
                   start=True, stop=False)
                mm(out=p3[:], lhsT=ones_row[:], rhs=W["ne_b3"][:],
                   start=False, stop=True)
                h_sb = sb2.tile([P, L], fr, tag="enc_h")
                nc.vector.tensor_copy(h_sb[:], p3[:])
                nc.gpsimd.dma_start(out=h_A[ds(c * 512 + j * P, P), :],
                                    in_=h_sb[:])

        # ---- edge encoder -> eblk, one block per iteration ----
        with tc.For_i(0, BPC, 1) as b:
            ea_t = sbe.tile([P, E_blk], fr, tag="ebig")
            nc.gpsimd.dma_start(out=ea_t[:EI, :], in_=eafm_d[:, ts(b, E_blk)])
            e_all = sbe.tile([P, E_blk], fr, tag="ebig")
            for (t0, k) in strips:
                w = k * P
                cs = slice(t0 * P, t0 * P + w)
                p1 = ps_b.tile([P, w], f32, tag="mm_big")
                mm(out=p1[:], lhsT=r(W["ee_W1"][:]), rhs=r(ea_t[:EI, cs]),
                   start=True, stop=True)
                a1 = sb2.tile([P, w], fr, tag="ee_a1")
                nc.scalar.activation(out=a1[:], in_=p1[:], func=Relu,
                                     bias=W["ee_b1"][:, :1])
                p2 = ps_b.tile([P, w], f32, tag="mm_big")
                mm(out=p2[:], lhsT=r(W["ee_W2"][:]), rhs=r(a1[:]),
                   start=True, stop=True)
                a2 = sb2.tile([P, w], fr, tag="ee_a2")
                nc.scalar.activation(out=a2[:], in_=p2[:], func=Relu,
                                     bias=W["ee_b2"][:, :1])
                p3 = ps_b.tile([P, w], f32, tag="mm_big")
                mm(out=p3[:], lhsT=r(W["ee_W3"][:]), rhs=r(a2[:]),
                   start=True, stop=True)
                nc.vector.tensor_scalar_add(e_all[:, cs], p3[:],
                                            W["ee_b3"][:, :1])
            nc.gpsimd.dma_start(out=eblk[ts(b, P), :], in_=e_all[:])

        # ---- message passing steps ----
        for s in range(S):
            h_r = read_buf[s]
            b3p = ps_s.tile([P, L], f32, tag="mm_small")
            mm(out=b3p[:], lhsT=ones_row[:], rhs=W[f"pe_b3_{s}"][:],
               start=True, stop=True)
            b3b = wp.tile([P, L], f32, tag=f"b3b_{s}")
            nc.vector.tensor_copy(b3b[:], b3p[:])

            eng_be = nc.sync if s % 2 == 0 else nc.scalar
            eng_bo = nc.scalar if s % 2 == 0 else nc.sync
            with tc.For_i(0, BPC, 1) as b:
                blob_t = sbm.tile([P, C], f32, tag="blob_t")
                eng_be.dma_start(out=blob_t[:], in_=blob_d[ts(b, P), :])
                dlr_t = sbm.tile([1, E_blk], fr, tag="dlr_t")
                eng_be.dma_start(out=dlr_t[:], in_=dlocr_d[ds(b, 1), :])
                eb_t = sbe.tile([P, E_blk], fr, tag="ebig")
                eng_be.dma_start(out=eb_t[:], in_=eblk[ts(b, P), :])
                hblk_t = sb.tile([P, L], fr, tag="hblk")
                nc.gpsimd.indirect_dma_start(
                    out=hblk_t[:], out_offset=None, in_=h_r[:, :],
                    in_offset=bass.IndirectOffsetOnAxis(
                        ap=blob_t[:, 2 + T:3 + T].bitcast(i32), axis=0))
                hblk = hblk_t[:]

                agg_p = ps_a.tile([P, L], f32, tag="agg")
                first = True
                for (t0, k) in strips:
                    w = k * P
                    hs_fm = sb.tile([P, w], fr, tag="hs_fm")
                    for j in range(k):
                        t = t0 + j
                        g1 = sbga.tile([P, L], fr, tag="g_all")
                        nc.gpsimd.indirect_dma_start(
                            out=g1[:], out_offset=None, in_=h_r[:, :],
                            in_offset=bass.IndirectOffsetOnAxis(
                                ap=blob_t[:, 3 + T + t:4 + T + t].bitcast(i32),
                                axis=0))
                        tp1 = ps_s.tile([P, P], fr, tag="mm_small")
                        nc.tensor.transpose(out=tp1[:], in_=g1[:],
                                            identity=identity_r[:])
                        nc.vector.tensor_copy(hs_fm[:, j * P:(j + 1) * P],
                                              tp1[:])
                    # h[dst] via one-hot: O_T[n, e] = (dloc[e] == n)
                    dlb = ps_b.tile([P, w], f32, tag="mm_big")
                    mm(out=dlb[:], lhsT=ones_row_r[:],
                       rhs=dlr_t[:1, t0 * P:t0 * P + w], start=True, stop=True)
                    O_T = sb.tile([P, w], fr, tag="O_T")
                    nc.vector.tensor_tensor(out=O_T[:], in0=dlb[:],
                                            in1=iotac_b[:, :w], op=AO.is_equal)
                    hdp = ps_b.tile([P, w], f32, tag="mm_big")
                    mm(out=hdp[:], lhsT=r(hblk), rhs=r(O_T[:]),
                       start=True, stop=True)
                    hd_fm = sb.tile([P, w], fr, tag="hd_fm")
                    nc.scalar.activation(out=hd_fm[:], in_=hdp[:],
                                         func=mybir.ActivationFunctionType.Copy)

                    h1p = ps_b.tile([P, w], f32, tag="mm_big")
                    mm(out=h1p[:], lhsT=r(W[f"pe_W1_{s}_0"][:]),
                       rhs=r(hd_fm[:]), start=True, stop=False)
                    mm(out=h1p[:], lhsT=r(W[f"pe_W1_{s}_1"][:]),
                       rhs=r(hs_fm[:]), start=False, stop=False)
                    mm(out=h1p[:], lhsT=r(W[f"pe_W1_{s}_2"][:]),
                       rhs=r(eb_t[:, t0 * P:t0 * P + w]),
                       start=False, stop=True)
                    a1 = sb.tile([P, w], fr, tag="pe_a1")
                    nc.scalar.activation(out=a1[:], in_=h1p[:], func=Relu,
                                         bias=W[f"pe_b1_{s}"][:, :1])
                    h2p = ps_b.tile([P, w], f32, tag="mm_big")
                    mm(out=h2p[:], lhsT=r(W[f"pe_W2_{s}"][:]), rhs=r(a1[:]),
                       start=True, stop=True)
                    a2 = sb.tile([P, w], fr, tag="pe_a2")
                    nc.scalar.activation(out=a2[:], in_=h2p[:], func=Relu,
                                         bias=W[f"pe_b2_{s}"][:, :1])
                    for j in range(k):
                        t = t0 + j
                        mp = ps_s.tile([P, P], f32, tag="mm_small")
                        mm(out=mp[:], lhsT=a2[:, j * P:(j + 1) * P],
                           rhs=W[f"pe_W3_{s}"][:], start=True, stop=True)
                        msb = sbg.tile([P, P], f32, tag="msb")
                        nc.vector.tensor_copy(msb[:], mp[:])
                        O_j = sbg.tile([P, P], f32, tag="O_j")
                        nc.vector.tensor_tensor(
                            out=O_j[:],
                            in0=blob_t[:, 2 + t:3 + t].to_broadcast([P, P])[:],
                            in1=iota_f[:], op=AO.is_equal)
                        mm(out=agg_p[:], lhsT=O_j[:], rhs=msb[:],
                           start=first, stop=(t == T - 1))
                        first = False
                # agg = inv_deg * sum + mask * b3
                agg_sb = sb.tile([P, L], f32, tag="agg_sb")
                nc.vector.tensor_scalar(out=agg_sb[:], in0=agg_p[:],
                                        scalar1=blob_t[:, 0:1], scalar2=None,
                                        op0=AO.mult)
                b3m = sb.tile([P, L], f32, tag="b3m")
                nc.vector.tensor_scalar(out=b3m[:], in0=b3b[:],
                                        scalar1=blob_t[:, 1:2], scalar2=None,
                                        op0=AO.mult)
                nc.vector.tensor_tensor(out=agg_sb[:], in0=agg_sb[:],
                                        in1=b3m[:], op=AO.add)
                # node update MLP
                tr1 = ps_s.tile([P, P], fr, tag="mm_small")
                nc.tensor.transpose(out=tr1[:], in_=hblk,
                                    identity=identity_r[:])
                hfm = sb.tile([P, P], f32, tag="hfm")
                nc.vector.tensor_copy(hfm[:], tr1[:])
                tr2 = ps_s.tile([P, P], f32, tag="mm_small")
                nc.tensor.transpose(out=tr2[:], in_=agg_sb[:],
                                    identity=identity[:])
                afm = sb.tile([P, P], f32, tag="afm")
                nc.vector.tensor_copy(afm[:], tr2[:])
                n1p = ps_s.tile([P, P], f32, tag="mm_small")
                mm(out=n1p[:], lhsT=W[f"pn_W1_{s}_0"][:], rhs=hfm[:],
                   start=True, stop=False)
                mm(out=n1p[:], lhsT=W[f"pn_W1_{s}_1"][:], rhs=afm[:],
                   start=False, stop=True)
                n1 = sb.tile([P, P], f32, tag="n1")
                nc.scalar.activation(out=n1[:], in_=n1p[:], func=Relu,
                                     bias=W[f"pn_b1_{s}"][:, :1])
                n2p = ps_s.tile([P, P], f32, tag="mm_small")
                mm(out=n2p[:], lhsT=W[f"pn_W2_{s}"][:], rhs=n1[:],
                   start=True, stop=True)
                n2 = sb.tile([P, P], f32, tag="n2")
                nc.scalar.activation(out=n2[:], in_=n2p[:], func=Relu,
                                     bias=W[f"pn_b2_{s}"][:, :1])
                n3p = ps_s.tile([P, P], f32, tag="mm_small")
                mm(out=n3p[:], lhsT=n2[:], rhs=W[f"pn_W3_{s}"][:],
                   start=True, stop=False)
                mm(out=n3p[:], lhsT=ones_row[:], rhs=W[f"pn_b3_{s}"][:],
                   start=False, stop=True)
                hnew = sb.tile([P, L], fr, tag="hnew")
                nc.vector.tensor_tensor(out=hnew[:], in0=n3p[:], in1=hblk,
                                        op=AO.add)
                eng_bo.dma_start(out=h_own[ts(b, P), :], in_=hnew[:])
            if write_buf[s] is not None:
                nc.gpsimd.collective_compute(
                    "AllGather", mybir.AluOpType.bypass,
                    replica_groups=[list(range(n_cores))],
                    ins=[h_own[:, :]], outs=[write_buf[s][:, :]])

        # ---- decoder ----
        with tc.For_i(0, BPC, 1) as b:
            hblk = sb.tile([P, L], fr, tag="dec_hblk")
            nc.gpsimd.dma_start(out=hblk[:], in_=h_own[ts(b, P), :])
            tr = ps_s.tile([P, P], fr, tag="mm_small")
            nc.tensor.transpose(out=tr[:], in_=hblk[:],
                                identity=identity_r[:])
            hfm = sb.tile([P, P], f32, tag="dec_hfm")
            nc.vector.tensor_copy(hfm[:], tr[:])
            d1p = ps_s.tile([P, P], f32, tag="mm_small")
            mm(out=d1p[:], lhsT=W["de_W1"][:], rhs=hfm[:], start=True, stop=True)
            d1 = sb.tile([P, P], f32, tag="d1")
            nc.scalar.activation(out=d1[:], in_=d1p[:], func=Relu,
                                 bias=W["de_b1"][:, :1])
            d2p = ps_s.tile([P, P], f32, tag="mm_small")
            mm(out=d2p[:], lhsT=W["de_W2"][:], rhs=d1[:], start=True, stop=True)
            d2 = sb.tile([P, P], f32, tag="d2")
            nc.scalar.activation(out=d2[:], in_=d2p[:], func=Relu,
                                 bias=W["de_b2"][:, :1])
            dp = ps_s.tile([P, OD], f32, tag="mm_small")
            mm(out=dp[:], lhsT=d2[:], rhs=W["de_W3"][:], start=True, stop=False)
            mm(out=dp[:], lhsT=ones_row[:], rhs=W["de_b3"][:],
               start=False, stop=True)
            osb = sb.tile([P, OD], f32, tag="osb")
            nc.vector.tensor_copy(osb[:], dp[:])
            nc.gpsimd.dma_start(out=out_d[ts(b, P), :], in_=osb[:])

    nc.finalize()
    return nc


def _ensure_ntff_hook():
    """Register the axon NTFF profiling hook if the image lacks
    antenv.axon_hooks (replicates trn_boot's ctypes wiring)."""
    import sys
    import types
    try:
        import antenv.axon_hooks  # noqa: F401
        return
    except ImportError:
        pass
    import contextlib
    import ctypes
    import antenv

    m = types.ModuleType("antenv.axon_hooks")
    state = {"hook": None, "tried": False}

    def set_axon_ntff_profile_hook(hook):
        state["hook"] = hook

    def _make_hook(so_path="/opt/axon/libaxon_pjrt.so"):
        lib = ctypes.CDLL(so_path)
        if not hasattr(lib, "axon_start_nrt_profile"):
            return None
        lib.axon_start_nrt_profile.argtypes = [
            ctypes.POINTER(ctypes.c_int64), ctypes.c_size_t]
        lib.axon_start_nrt_profile.restype = ctypes.c_int64
        lib.axon_stop_nrt_profile.argtypes = [ctypes.c_char_p]
        lib.axon_stop_nrt_profile.restype = ctypes.c_int64

        @contextlib.contextmanager
        def _hook(output_dir, device_ids):
            import jax
            jax.devices()
            if device_ids:
                ids = (ctypes.c_int64 * len(device_ids))(*device_ids)
                rc = lib.axon_start_nrt_profile(ids, len(device_ids))
            else:
                rc = lib.axon_start_nrt_profile(None, 0)
            if rc != 0:
                raise RuntimeError(f"axon_start_nrt_profile rc={rc}")
            try:
                yield
            finally:
                n = lib.axon_stop_nrt_profile(str(output_dir).encode())
                print(f"ntff profile: {n} file(s) written to {output_dir}")

        return _hook

    def get_axon_ntff_profile_hook():
        if state["hook"] is None and not state["tried"]:
            state["tried"] = True
            try:
                state["hook"] = _make_hook()
            except OSError:
                state["hook"] = None
        return state["hook"]

    m.set_axon_ntff_profile_hook = set_axon_ntff_profile_hook
    m.get_axon_ntff_profile_hook = get_axon_ntff_profile_hook
    sys.modules["antenv.axon_hooks"] = m
    antenv.axon_hooks = m


def kernel(**inputs):
    n_cores = 8
    params, in_maps = prep_host(inputs, n_cores)
    nc = build_program(params, debug=False)

    from concourse.bass_utils import run_bass_kernel_spmd
    import time
    trace = bool(int(os.environ.get("KERNEL_TRACE", "0")))
    if trace:
        try:
            _ensure_ntff_hook()
        except Exception:
            pass
    t0 = time.time()
    try:
        res = run_bass_kernel_spmd(nc, in_maps, list(range(n_cores)),
                                   trace=trace)
    except ModuleNotFoundError:
        res = run_bass_kernel_spmd(nc, in_maps, list(range(n_cores)),
                                   trace=False)
    LAST["wall_s"] = time.time() - t0
    LAST["exec_time_ns"] = getattr(res, "exec_time_ns", None)
    LAST["profile_json"] = getattr(res, "profile_json", None)
    LAST["params"] = params
    out = np.concatenate([r["out"] for r in res.results], axis=0)
    return np.ascontiguousarray(out[:params["N"]].astype(np.float32))

